# revision 10
# baseline (speedup 1.0000x reference)
"""DeepSeek-V3 TopK router kernel for 8 Trainium2 NeuronCores.

Strategy (data/sequence parallel per sharding hint):
 - Shard the 16384 tokens across 8 cores (2048 tokens each); replicate the
   router weight + bias.
 - Host-side layout prep: x and w are pre-transposed so the contraction dim
   (hidden) lands on SBUF partitions; total DMA bytes are unchanged.
 - Device per core: 56 accumulating fp32r matmuls per 128-token tile
   (lhsT = x^T chunk [128h,128t] stationary, rhs = w^T chunk [128h,256e]
   moving, PSUM [128t,256e]); sigmoid on ScalarE; group top-2 / top-4 and
   masked top-8 via the DVE max8/max_index ops; weight gather via fused
   scalar_tensor_tensor (eq-mask * scores, reduced); normalize with DVE
   reciprocal.
"""

import sys

for _p in ("/opt/trn_rl_repo", "/root/.axon_site/_ro/trn_rl_repo"):
    if _p not in sys.path:
        sys.path.append(_p)

import numpy as np

import concourse.bass as bass
import concourse.bacc as bacc
import concourse.mybir as mybir
from concourse.bass import ds
from concourse.tile import TileContext
from concourse.bass_utils import run_bass_kernel_spmd

# Problem constants (hardcoded per contract)
T = 16384          # batch*seq = 4*4096
H = 7168           # hidden
E = 256            # experts
N_CORES = 8
TC = T // N_CORES  # tokens per core = 2048
KH = H // 128      # 56 contraction chunks
NG = 8             # expert groups
GS = E // NG       # group size = 32
TOPK = 8
SCALE = 2.5
TGROUP = 256       # tokens per DMA group (2 matmul subtiles)

F32 = mybir.dt.float32
F32R = mybir.dt.float32r
U32 = mybir.dt.uint32


def build_nc(tc_tokens: int = TC, tgroup: int = TGROUP, debug: bool = False) -> bass.Bass:
    nc = bacc.Bacc(trn_type="TRN2")
    xt = nc.declare_dram_parameter("xt", [H, tc_tokens], F32, isOutput=False)
    wt = nc.declare_dram_parameter("wt", [H, E], F32, isOutput=False)
    biasb = nc.declare_dram_parameter("biasb", [128, E], F32, isOutput=False)
    iotab = nc.declare_dram_parameter("iotab", [128, E], F32, isOutput=False)
    out_logits = nc.declare_dram_parameter("out_logits", [tc_tokens, E], F32, isOutput=True)
    out_idx = nc.declare_dram_parameter("out_idx", [tc_tokens, TOPK], U32, isOutput=True)
    out_w = nc.declare_dram_parameter("out_w", [tc_tokens, TOPK], F32, isOutput=True)
    if debug:
        out_w8 = nc.declare_dram_parameter("out_w8", [tc_tokens, TOPK], F32, isOutput=True)
        out_den = nc.declare_dram_parameter("out_den", [tc_tokens, 2], F32, isOutput=True)
        out_eqs = nc.declare_dram_parameter("out_eqs", [tc_tokens, E], F32, isOutput=True)

    n_groups = tc_tokens // tgroup
    subs = tgroup // 128

    with TileContext(nc) as tc:
        with (
            tc.tile_pool(name="const", bufs=1) as cpool,
            tc.tile_pool(name="xs", bufs=2) as xpool,
            tc.tile_pool(name="ps", bufs=4, space="PSUM") as ppool,
            tc.tile_pool(name="work", bufs=2) as spool,
        ):
            wt_sb = cpool.tile([128, KH, E], F32)
            nc.sync.dma_start(out=wt_sb, in_=wt[:, :].rearrange("(k p) e -> p k e", p=128))
            bias_sb = cpool.tile([128, E], F32)
            nc.sync.dma_start(out=bias_sb, in_=biasb[:, :])
            iota_sb = cpool.tile([128, E], F32)
            nc.sync.dma_start(out=iota_sb, in_=iotab[:, :])
            # Dummy ldweights consuming wt_sb: folds the weight-DMA wait into
            # the PE's clock so the first real matmul carries only the x-tile
            # wait (the LDWEIGHTS ISA slot fits a single sync wait). The real
            # matmuls self-load their stationary operands, overwriting this.
            nc.tensor.ldweights(wt_sb[:, 0, 0:64].bitcast(mybir.dt.bfloat16))

            for g in range(n_groups):
                xg = xpool.tile([128, KH, tgroup], F32, tag="xg")
                nc.sync.dma_start(
                    out=xg,
                    in_=xt[:, ds(g * tgroup, tgroup)].rearrange("(k p) t -> p k t", p=128),
                )
                for sub in range(subs):
                    t0 = g * tgroup + sub * 128
                    lg = ppool.tile([128, E], F32, tag="lg")
                    for k in range(KH):
                        nc.tensor.matmul(
                            lg,
                            xg[:, k, ds(sub * 128, 128)],
                            wt_sb[:, k, :],
                            start=(k == 0),
                            stop=(k == KH - 1),
                        )
                    # router logits: PSUM -> SBUF -> DRAM
                    lsb = spool.tile([128, E], F32, tag="lsb")
                    nc.scalar.copy(lsb, lg)
                    nc.scalar.dma_start(out=out_logits[ds(t0, 128), :], in_=lsb)
                    scores = spool.tile([128, E], F32, tag="scores")
                    nc.scalar.activation(scores, lg, mybir.ActivationFunctionType.Sigmoid)
                    s4c = spool.tile([128, E], F32, tag="s4c")
                    nc.vector.tensor_add(s4c, scores, bias_sb)

                    # per-group top-2 -> group score
                    gtop = spool.tile([128, NG, 8], F32, tag="gtop")
                    for j in range(NG):
                        nc.vector.max(out=gtop[:, j, :], in_=s4c[:, ds(j * GS, GS)])
                    gsum = spool.tile([128, NG], F32, tag="gsum")
                    nc.vector.tensor_add(gsum, gtop[:, :, 0], gtop[:, :, 1])
                    # top-4 groups -> mask
                    gs8 = spool.tile([128, 8], F32, tag="gs8")
                    nc.vector.max(out=gs8, in_=gsum)
                    gmask = spool.tile([128, NG], F32, tag="gmask")
                    nc.vector.tensor_scalar(
                        gmask, gsum, gs8[:, 3:4], None, op0=mybir.AluOpType.is_ge
                    )
                    masked = spool.tile([128, E], F32, tag="masked")
                    for j in range(NG):
                        nc.vector.tensor_scalar_mul(
                            masked[:, ds(j * GS, GS)], s4c[:, ds(j * GS, GS)], gmask[:, j : j + 1]
                        )
                    # masked top-8 with indices
                    top8v = spool.tile([128, 8], F32, tag="top8v")
                    nc.vector.max(out=top8v, in_=masked)
                    idx8 = spool.tile([128, 8], U32, tag="idx8")
                    nc.vector.max_index(idx8, top8v, masked)
                    # gather scores at the top-8 positions by POSITION
                    # (iota == idx_k is a guaranteed one-hot; value-matching
                    # breaks when two experts tie bit-exactly)
                    idx8f = spool.tile([128, TOPK], F32, tag="idx8f")
                    nc.vector.tensor_copy(idx8f, idx8)
                    w8 = spool.tile([128, TOPK], F32, tag="w8")
                    eqs = spool.tile([128, E], F32, tag="eqs")
                    for k in range(TOPK):
                        nc.vector.scalar_tensor_tensor(
                            out=eqs,
                            in0=iota_sb,
                            scalar=idx8f[:, k : k + 1],
                            in1=scores,
                            op0=mybir.AluOpType.is_equal,
                            op1=mybir.AluOpType.mult,
                            accum_out=w8[:, k : k + 1],
                        )
                    denom = spool.tile([128, 1], F32, tag="denom")
                    nc.vector.reduce_sum(denom, w8, axis=mybir.AxisListType.X)
                    rden = spool.tile([128, 1], F32, tag="rden")
                    nc.vector.reciprocal(rden, denom)
                    wout = spool.tile([128, TOPK], F32, tag="wout")
                    nc.vector.tensor_scalar(
                        wout, w8, rden, SCALE,
                        op0=mybir.AluOpType.mult, op1=mybir.AluOpType.mult,
                    )
                    nc.scalar.dma_start(out=out_idx[ds(t0, 128), :], in_=idx8)
                    nc.scalar.dma_start(out=out_w[ds(t0, 128), :], in_=wout)
                    if debug:
                        nc.scalar.dma_start(out=out_w8[ds(t0, 128), :], in_=w8)
                        nc.scalar.dma_start(out=out_den[ds(t0, 128), 0:1], in_=denom)
                        nc.scalar.dma_start(out=out_den[ds(t0, 128), 1:2], in_=rden)
                        nc.scalar.dma_start(out=out_eqs[ds(t0, 128), :], in_=eqs)
    nc.finalize()
    return nc


F16 = mybir.dt.float16
XSCALE = 1024.0       # 2**10: keeps x_lo out of fp16 denormals
WSCALE = 8192.0       # 2**13: keeps w_lo out of fp16 denormals
DESCALE = 1.0 / (XSCALE * WSCALE)  # 2**-23, exact power of two


def build_nc_f16(tc_tokens: int = TC, tgroup: int = TGROUP, debug: bool = False) -> bass.Bass:
    """fp16 hi/lo 3-pass matmul variant: x' = x*2^10 = xhi+xlo (fp16),
    w' = w*2^13 = whi+wlo (fp16); logits' = xhi*whi + xlo*whi + xhi*wlo
    accumulated in fp32 PSUM; descale by 2^-23 (exact) on the way out.
    Error ~2^-21 per product: below fp32 summation noise, at 3x bf16-rate
    PE cost instead of 4x for native fp32."""
    nc = bacc.Bacc(trn_type="TRN2")
    xhi = nc.declare_dram_parameter("xhi", [H, tc_tokens], F16, isOutput=False)
    xlo = nc.declare_dram_parameter("xlo", [H, tc_tokens], F16, isOutput=False)
    whi = nc.declare_dram_parameter("whi", [H, E], F16, isOutput=False)
    wlo = nc.declare_dram_parameter("wlo", [H, E], F16, isOutput=False)
    biasb = nc.declare_dram_parameter("biasb", [128, E], F32, isOutput=False)
    iotab = nc.declare_dram_parameter("iotab", [128, E], F32, isOutput=False)
    out_logits = nc.declare_dram_parameter("out_logits", [tc_tokens, E], F32, isOutput=True)
    out_idx = nc.declare_dram_parameter("out_idx", [tc_tokens, TOPK], U32, isOutput=True)
    out_w = nc.declare_dram_parameter("out_w", [tc_tokens, TOPK], F32, isOutput=True)

    n_groups = tc_tokens // tgroup
    subs = tgroup // 128

    with TileContext(nc) as tc:
        with (
            tc.tile_pool(name="const", bufs=1) as cpool,
            tc.tile_pool(name="xs", bufs=2) as xpool,
            tc.tile_pool(name="ps", bufs=4, space="PSUM") as ppool,
            tc.tile_pool(name="work", bufs=2) as spool,
        ):
            whi_sb = cpool.tile([128, KH, E], F16)
            nc.sync.dma_start(out=whi_sb, in_=whi[:, :].rearrange("(k p) e -> p k e", p=128))
            wlo_sb = cpool.tile([128, KH, E], F16)
            nc.sync.dma_start(out=wlo_sb, in_=wlo[:, :].rearrange("(k p) e -> p k e", p=128))
            bias_sb = cpool.tile([128, E], F32)
            nc.sync.dma_start(out=bias_sb, in_=biasb[:, :])
            iota_sb = cpool.tile([128, E], F32)
            nc.sync.dma_start(out=iota_sb, in_=iotab[:, :])
            # fold the two weight-DMA waits into the PE clock up front
            # (LDWEIGHTS fits a single sync wait)
            nc.tensor.ldweights(whi_sb[:, 0, 0:64])
            nc.tensor.ldweights(wlo_sb[:, 0, 0:64])

            for g in range(n_groups):
                xghi = xpool.tile([128, KH, tgroup], F16, tag="xghi")
                nc.sync.dma_start(
                    out=xghi,
                    in_=xhi[:, ds(g * tgroup, tgroup)].rearrange("(k p) t -> p k t", p=128),
                )
                xglo = xpool.tile([128, KH, tgroup], F16, tag="xglo")
                nc.sync.dma_start(
                    out=xglo,
                    in_=xlo[:, ds(g * tgroup, tgroup)].rearrange("(k p) t -> p k t", p=128),
                )
                for sub in range(subs):
                    t0 = g * tgroup + sub * 128
                    ts_ = ds(sub * 128, 128)
                    lg = ppool.tile([128, E], F32, tag="lg")
                    for k in range(KH):
                        nc.tensor.matmul(lg, xghi[:, k, ts_], whi_sb[:, k, :],
                                         start=(k == 0), stop=False)
                        nc.tensor.matmul(lg, xghi[:, k, ts_], wlo_sb[:, k, :],
                                         start=False, stop=False)
                        nc.tensor.matmul(lg, xglo[:, k, ts_], whi_sb[:, k, :],
                                         start=False, stop=(k == KH - 1))
                    # descaled router logits: PSUM -> SBUF -> DRAM
                    lsb = spool.tile([128, E], F32, tag="lsb")
                    nc.scalar.mul(lsb, lg, DESCALE)
                    nc.scalar.dma_start(out=out_logits[ds(t0, 128), :], in_=lsb)
                    scores = spool.tile([128, E], F32, tag="scores")
                    nc.scalar.activation(scores, lg, mybir.ActivationFunctionType.Sigmoid,
                                         scale=DESCALE)
                    s4c = spool.tile([128, E], F32, tag="s4c")
                    nc.vector.tensor_add(s4c, scores, bias_sb)

                    gtop = spool.tile([128, NG, 8], F32, tag="gtop")
                    for j in range(NG):
                        nc.vector.max(out=gtop[:, j, :], in_=s4c[:, ds(j * GS, GS)])
                    gsum = spool.tile([128, NG], F32, tag="gsum")
                    nc.vector.tensor_add(gsum, gtop[:, :, 0], gtop[:, :, 1])
                    gs8 = spool.tile([128, 8], F32, tag="gs8")
                    nc.vector.max(out=gs8, in_=gsum)
                    gmask = spool.tile([128, NG], F32, tag="gmask")
                    nc.vector.tensor_scalar(
                        gmask, gsum, gs8[:, 3:4], None, op0=mybir.AluOpType.is_ge
                    )
                    masked = spool.tile([128, E], F32, tag="masked")
                    for j in range(NG):
                        nc.vector.tensor_scalar_mul(
                            masked[:, ds(j * GS, GS)], s4c[:, ds(j * GS, GS)], gmask[:, j : j + 1]
                        )
                    top8v = spool.tile([128, 8], F32, tag="top8v")
                    nc.vector.max(out=top8v, in_=masked)
                    idx8 = spool.tile([128, 8], U32, tag="idx8")
                    nc.vector.max_index(idx8, top8v, masked)
                    idx8f = spool.tile([128, TOPK], F32, tag="idx8f")
                    nc.vector.tensor_copy(idx8f, idx8)
                    w8 = spool.tile([128, TOPK], F32, tag="w8")
                    eqs = spool.tile([128, E], F32, tag="eqs")
                    for k in range(TOPK):
                        nc.vector.scalar_tensor_tensor(
                            out=eqs,
                            in0=iota_sb,
                            scalar=idx8f[:, k : k + 1],
                            in1=scores,
                            op0=mybir.AluOpType.is_equal,
                            op1=mybir.AluOpType.mult,
                            accum_out=w8[:, k : k + 1],
                        )
                    denom = spool.tile([128, 1], F32, tag="denom")
                    nc.vector.reduce_sum(denom, w8, axis=mybir.AxisListType.X)
                    rden = spool.tile([128, 1], F32, tag="rden")
                    nc.vector.reciprocal(rden, denom)
                    wout = spool.tile([128, TOPK], F32, tag="wout")
                    nc.vector.tensor_scalar(
                        wout, w8, rden, SCALE,
                        op0=mybir.AluOpType.mult, op1=mybir.AluOpType.mult,
                    )
                    nc.scalar.dma_start(out=out_idx[ds(t0, 128), :], in_=idx8)
                    nc.scalar.dma_start(out=out_w[ds(t0, 128), :], in_=wout)
    nc.finalize()
    return nc


def make_in_maps_f16(hidden_states, weight, e_score_correction_bias):
    x = np.ascontiguousarray(np.asarray(hidden_states, dtype=np.float32)).reshape(T, H)
    w = np.asarray(weight, dtype=np.float32)
    b = np.asarray(e_score_correction_bias, dtype=np.float32)
    ws = np.ascontiguousarray(w.T) * np.float32(WSCALE)
    whi = ws.astype(np.float16)
    wlo = (ws - whi.astype(np.float32)).astype(np.float16)
    biasb = np.ascontiguousarray(np.broadcast_to(b[None, :], (128, E)))
    iotab = np.ascontiguousarray(
        np.broadcast_to(np.arange(E, dtype=np.float32)[None, :], (128, E)))
    xt_full = x.T  # view
    in_maps = []
    for c in range(N_CORES):
        xs = np.ascontiguousarray(xt_full[:, c * TC : (c + 1) * TC]) * np.float32(XSCALE)
        xhi = xs.astype(np.float16)
        xlo = (xs - xhi.astype(np.float32)).astype(np.float16)
        in_maps.append({"xhi": xhi, "xlo": xlo, "whi": whi, "wlo": wlo,
                        "biasb": biasb, "iotab": iotab})
    return in_maps


_NC = None


def _get_nc():
    global _NC
    if _NC is None:
        _NC = build_nc_f16()
    return _NC


def make_in_maps(hidden_states, weight, e_score_correction_bias):
    x = np.ascontiguousarray(np.asarray(hidden_states, dtype=np.float32)).reshape(T, H)
    w = np.asarray(weight, dtype=np.float32)
    b = np.asarray(e_score_correction_bias, dtype=np.float32)
    wt = np.ascontiguousarray(w.T)
    biasb = np.ascontiguousarray(np.broadcast_to(b[None, :], (128, E)))
    iotab = np.ascontiguousarray(np.broadcast_to(np.arange(E, dtype=np.float32)[None, :], (128, E)))
    xt_full = x.T  # view
    in_maps = []
    for c in range(N_CORES):
        xt_c = np.ascontiguousarray(xt_full[:, c * TC : (c + 1) * TC])
        in_maps.append({"xt": xt_c, "wt": wt, "biasb": biasb, "iotab": iotab})
    return in_maps


def assemble(results):
    logits = np.concatenate([results[c]["out_logits"] for c in range(N_CORES)], axis=0)
    idx = np.concatenate([results[c]["out_idx"] for c in range(N_CORES)], axis=0).astype(np.int32)
    wts = np.concatenate([results[c]["out_w"] for c in range(N_CORES)], axis=0)
    return idx, wts, logits


def kernel(hidden_states, weight, e_score_correction_bias):
    nc = _get_nc()
    in_maps = make_in_maps_f16(hidden_states, weight, e_score_correction_bias)
    res = run_bass_kernel_spmd(nc, in_maps, list(range(N_CORES)))
    return assemble(res.results)


# revision 11
# speedup vs baseline: 1.2269x; 1.2269x over previous
"""DeepSeek-V3 TopK router kernel for 8 Trainium2 NeuronCores.

Strategy (data/sequence parallel per sharding hint):
 - Shard the 16384 tokens across 8 cores (2048 tokens each); replicate the
   router weight + bias.
 - Host-side layout prep: x and w are pre-transposed so the contraction dim
   (hidden) lands on SBUF partitions; total DMA bytes are unchanged.
 - Device per core: 56 accumulating fp32r matmuls per 128-token tile
   (lhsT = x^T chunk [128h,128t] stationary, rhs = w^T chunk [128h,256e]
   moving, PSUM [128t,256e]); sigmoid on ScalarE; group top-2 / top-4 and
   masked top-8 via the DVE max8/max_index ops; weight gather via fused
   scalar_tensor_tensor (eq-mask * scores, reduced); normalize with DVE
   reciprocal.
"""

import sys

for _p in ("/opt/trn_rl_repo", "/root/.axon_site/_ro/trn_rl_repo"):
    if _p not in sys.path:
        sys.path.append(_p)

import numpy as np

import concourse.bass as bass
import concourse.bacc as bacc
import concourse.mybir as mybir
from concourse.bass import ds
from concourse.tile import TileContext
from concourse.bass_utils import run_bass_kernel_spmd

# Problem constants (hardcoded per contract)
T = 16384          # batch*seq = 4*4096
H = 7168           # hidden
E = 256            # experts
N_CORES = 8
TC = T // N_CORES  # tokens per core = 2048
KH = H // 128      # 56 contraction chunks
NG = 8             # expert groups
GS = E // NG       # group size = 32
TOPK = 8
SCALE = 2.5
TGROUP = 256       # tokens per DMA group (2 matmul subtiles)

F32 = mybir.dt.float32
F32R = mybir.dt.float32r
U32 = mybir.dt.uint32


def build_nc(tc_tokens: int = TC, tgroup: int = TGROUP, debug: bool = False) -> bass.Bass:
    nc = bacc.Bacc(trn_type="TRN2")
    xt = nc.declare_dram_parameter("xt", [H, tc_tokens], F32, isOutput=False)
    wt = nc.declare_dram_parameter("wt", [H, E], F32, isOutput=False)
    biasb = nc.declare_dram_parameter("biasb", [128, E], F32, isOutput=False)
    iotab = nc.declare_dram_parameter("iotab", [128, E], F32, isOutput=False)
    out_logits = nc.declare_dram_parameter("out_logits", [tc_tokens, E], F32, isOutput=True)
    out_idx = nc.declare_dram_parameter("out_idx", [tc_tokens, TOPK], U32, isOutput=True)
    out_w = nc.declare_dram_parameter("out_w", [tc_tokens, TOPK], F32, isOutput=True)
    if debug:
        out_w8 = nc.declare_dram_parameter("out_w8", [tc_tokens, TOPK], F32, isOutput=True)
        out_den = nc.declare_dram_parameter("out_den", [tc_tokens, 2], F32, isOutput=True)
        out_eqs = nc.declare_dram_parameter("out_eqs", [tc_tokens, E], F32, isOutput=True)

    n_groups = tc_tokens // tgroup
    subs = tgroup // 128

    with TileContext(nc) as tc:
        with (
            tc.tile_pool(name="const", bufs=1) as cpool,
            tc.tile_pool(name="xs", bufs=2) as xpool,
            tc.tile_pool(name="ps", bufs=4, space="PSUM") as ppool,
            tc.tile_pool(name="work", bufs=2) as spool,
        ):
            wt_sb = cpool.tile([128, KH, E], F32)
            nc.sync.dma_start(out=wt_sb, in_=wt[:, :].rearrange("(k p) e -> p k e", p=128))
            bias_sb = cpool.tile([128, E], F32)
            nc.sync.dma_start(out=bias_sb, in_=biasb[:, :])
            iota_sb = cpool.tile([128, E], F32)
            nc.sync.dma_start(out=iota_sb, in_=iotab[:, :])
            # Dummy ldweights consuming wt_sb: folds the weight-DMA wait into
            # the PE's clock so the first real matmul carries only the x-tile
            # wait (the LDWEIGHTS ISA slot fits a single sync wait). The real
            # matmuls self-load their stationary operands, overwriting this.
            nc.tensor.ldweights(wt_sb[:, 0, 0:64].bitcast(mybir.dt.bfloat16))

            for g in range(n_groups):
                xg = xpool.tile([128, KH, tgroup], F32, tag="xg")
                nc.sync.dma_start(
                    out=xg,
                    in_=xt[:, ds(g * tgroup, tgroup)].rearrange("(k p) t -> p k t", p=128),
                )
                for sub in range(subs):
                    t0 = g * tgroup + sub * 128
                    lg = ppool.tile([128, E], F32, tag="lg")
                    for k in range(KH):
                        nc.tensor.matmul(
                            lg,
                            xg[:, k, ds(sub * 128, 128)],
                            wt_sb[:, k, :],
                            start=(k == 0),
                            stop=(k == KH - 1),
                        )
                    # router logits: PSUM -> SBUF -> DRAM
                    lsb = spool.tile([128, E], F32, tag="lsb")
                    nc.scalar.copy(lsb, lg)
                    nc.scalar.dma_start(out=out_logits[ds(t0, 128), :], in_=lsb)
                    scores = spool.tile([128, E], F32, tag="scores")
                    nc.scalar.activation(scores, lg, mybir.ActivationFunctionType.Sigmoid)
                    s4c = spool.tile([128, E], F32, tag="s4c")
                    nc.vector.tensor_add(s4c, scores, bias_sb)

                    # per-group top-2 -> group score
                    gtop = spool.tile([128, NG, 8], F32, tag="gtop")
                    for j in range(NG):
                        nc.vector.max(out=gtop[:, j, :], in_=s4c[:, ds(j * GS, GS)])
                    gsum = spool.tile([128, NG], F32, tag="gsum")
                    nc.vector.tensor_add(gsum, gtop[:, :, 0], gtop[:, :, 1])
                    # top-4 groups -> mask
                    gs8 = spool.tile([128, 8], F32, tag="gs8")
                    nc.vector.max(out=gs8, in_=gsum)
                    gmask = spool.tile([128, NG], F32, tag="gmask")
                    nc.vector.tensor_scalar(
                        gmask, gsum, gs8[:, 3:4], None, op0=mybir.AluOpType.is_ge
                    )
                    masked = spool.tile([128, E], F32, tag="masked")
                    for j in range(NG):
                        nc.vector.tensor_scalar_mul(
                            masked[:, ds(j * GS, GS)], s4c[:, ds(j * GS, GS)], gmask[:, j : j + 1]
                        )
                    # masked top-8 with indices
                    top8v = spool.tile([128, 8], F32, tag="top8v")
                    nc.vector.max(out=top8v, in_=masked)
                    idx8 = spool.tile([128, 8], U32, tag="idx8")
                    nc.vector.max_index(idx8, top8v, masked)
                    # gather scores at the top-8 positions by POSITION
                    # (iota == idx_k is a guaranteed one-hot; value-matching
                    # breaks when two experts tie bit-exactly)
                    idx8f = spool.tile([128, TOPK], F32, tag="idx8f")
                    nc.vector.tensor_copy(idx8f, idx8)
                    w8 = spool.tile([128, TOPK], F32, tag="w8")
                    eqs = spool.tile([128, E], F32, tag="eqs")
                    for k in range(TOPK):
                        nc.vector.scalar_tensor_tensor(
                            out=eqs,
                            in0=iota_sb,
                            scalar=idx8f[:, k : k + 1],
                            in1=scores,
                            op0=mybir.AluOpType.is_equal,
                            op1=mybir.AluOpType.mult,
                            accum_out=w8[:, k : k + 1],
                        )
                    denom = spool.tile([128, 1], F32, tag="denom")
                    nc.vector.reduce_sum(denom, w8, axis=mybir.AxisListType.X)
                    rden = spool.tile([128, 1], F32, tag="rden")
                    nc.vector.reciprocal(rden, denom)
                    wout = spool.tile([128, TOPK], F32, tag="wout")
                    nc.vector.tensor_scalar(
                        wout, w8, rden, SCALE,
                        op0=mybir.AluOpType.mult, op1=mybir.AluOpType.mult,
                    )
                    nc.scalar.dma_start(out=out_idx[ds(t0, 128), :], in_=idx8)
                    nc.scalar.dma_start(out=out_w[ds(t0, 128), :], in_=wout)
                    if debug:
                        nc.scalar.dma_start(out=out_w8[ds(t0, 128), :], in_=w8)
                        nc.scalar.dma_start(out=out_den[ds(t0, 128), 0:1], in_=denom)
                        nc.scalar.dma_start(out=out_den[ds(t0, 128), 1:2], in_=rden)
                        nc.scalar.dma_start(out=out_eqs[ds(t0, 128), :], in_=eqs)
    nc.finalize()
    return nc


F16 = mybir.dt.float16
XSCALE = 1024.0       # 2**10: keeps x_lo out of fp16 denormals
WSCALE = 8192.0       # 2**13: keeps w_lo out of fp16 denormals
DESCALE = 1.0 / (XSCALE * WSCALE)  # 2**-23, exact power of two


def build_nc_f16(tc_tokens: int = TC, tgroup: int = TGROUP, debug: bool = False) -> bass.Bass:
    """fp16 hi/lo 3-pass matmul variant: x' = x*2^10 = xhi+xlo (fp16),
    w' = w*2^13 = whi+wlo (fp16); logits' = xhi*whi + xlo*whi + xhi*wlo
    accumulated in fp32 PSUM; descale by 2^-23 (exact) on the way out.
    Error ~2^-21 per product: below fp32 summation noise, at 3x bf16-rate
    PE cost instead of 4x for native fp32."""
    nc = bacc.Bacc(trn_type="TRN2")
    xhi = nc.declare_dram_parameter("xhi", [H, tc_tokens], F16, isOutput=False)
    xlo = nc.declare_dram_parameter("xlo", [H, tc_tokens], F16, isOutput=False)
    whi = nc.declare_dram_parameter("whi", [H, E], F16, isOutput=False)
    wlo = nc.declare_dram_parameter("wlo", [H, E], F16, isOutput=False)
    biasb = nc.declare_dram_parameter("biasb", [128, E], F32, isOutput=False)
    iotab = nc.declare_dram_parameter("iotab", [128, E], F32, isOutput=False)
    out_logits = nc.declare_dram_parameter("out_logits", [tc_tokens, E], F32, isOutput=True)
    out_idx = nc.declare_dram_parameter("out_idx", [tc_tokens, TOPK], U32, isOutput=True)
    out_w = nc.declare_dram_parameter("out_w", [tc_tokens, TOPK], F32, isOutput=True)

    n_groups = tc_tokens // tgroup
    subs = tgroup // 128

    with TileContext(nc) as tc:
        with (
            tc.tile_pool(name="const", bufs=1) as cpool,
            tc.tile_pool(name="xs", bufs=2) as xpool,
            tc.tile_pool(name="ps", bufs=4, space="PSUM") as ppool,
            tc.tile_pool(name="work", bufs=2) as spool,
        ):
            whi_sb = cpool.tile([128, KH, E], F16)
            nc.sync.dma_start(out=whi_sb, in_=whi[:, :].rearrange("(k p) e -> p k e", p=128))
            wlo_sb = cpool.tile([128, KH, E], F16)
            nc.sync.dma_start(out=wlo_sb, in_=wlo[:, :].rearrange("(k p) e -> p k e", p=128))
            bias_sb = cpool.tile([128, E], F32)
            nc.sync.dma_start(out=bias_sb, in_=biasb[:, :])
            iota_sb = cpool.tile([128, E], F32)
            nc.sync.dma_start(out=iota_sb, in_=iotab[:, :])
            # fold the two weight-DMA waits into the PE clock up front
            # (LDWEIGHTS fits a single sync wait)
            nc.tensor.ldweights(whi_sb[:, 0, 0:64])
            nc.tensor.ldweights(wlo_sb[:, 0, 0:64])

            for g in range(n_groups):
                xghi = xpool.tile([128, KH, tgroup], F16, tag="xghi")
                nc.sync.dma_start(
                    out=xghi,
                    in_=xhi[:, ds(g * tgroup, tgroup)].rearrange("(k p) t -> p k t", p=128),
                )
                xglo = xpool.tile([128, KH, tgroup], F16, tag="xglo")
                nc.sync.dma_start(
                    out=xglo,
                    in_=xlo[:, ds(g * tgroup, tgroup)].rearrange("(k p) t -> p k t", p=128),
                )
                for sub in range(subs):
                    t0 = g * tgroup + sub * 128
                    ts_ = ds(sub * 128, 128)
                    lg = ppool.tile([128, E], F32, tag="lg")
                    for k in range(KH):
                        nc.tensor.matmul(lg, xghi[:, k, ts_], whi_sb[:, k, :],
                                         start=(k == 0), stop=False)
                        nc.tensor.matmul(lg, xghi[:, k, ts_], wlo_sb[:, k, :],
                                         start=False, stop=False)
                        nc.tensor.matmul(lg, xglo[:, k, ts_], whi_sb[:, k, :],
                                         start=False, stop=(k == KH - 1))
                    # descaled router logits: PSUM -> SBUF -> DRAM
                    lsb = spool.tile([128, E], F32, tag="lsb")
                    nc.scalar.mul(lsb, lg, DESCALE)
                    nc.scalar.dma_start(out=out_logits[ds(t0, 128), :], in_=lsb)
                    scores = spool.tile([128, E], F32, tag="scores")
                    nc.scalar.activation(scores, lg, mybir.ActivationFunctionType.Sigmoid,
                                         scale=DESCALE)
                    s4c = spool.tile([128, E], F32, tag="s4c")
                    nc.vector.tensor_add(s4c, scores, bias_sb)

                    gtop = spool.tile([128, NG, 8], F32, tag="gtop")
                    for j in range(NG):
                        nc.vector.max(out=gtop[:, j, :], in_=s4c[:, ds(j * GS, GS)])
                    gsum = spool.tile([128, NG], F32, tag="gsum")
                    nc.vector.tensor_add(gsum, gtop[:, :, 0], gtop[:, :, 1])
                    gs8 = spool.tile([128, 8], F32, tag="gs8")
                    nc.vector.max(out=gs8, in_=gsum)
                    gmask = spool.tile([128, NG], F32, tag="gmask")
                    nc.vector.tensor_scalar(
                        gmask, gsum, gs8[:, 3:4], None, op0=mybir.AluOpType.is_ge
                    )
                    masked = spool.tile([128, E], F32, tag="masked")
                    for j in range(NG):
                        nc.vector.tensor_scalar_mul(
                            masked[:, ds(j * GS, GS)], s4c[:, ds(j * GS, GS)], gmask[:, j : j + 1]
                        )
                    top8v = spool.tile([128, 8], F32, tag="top8v")
                    nc.vector.max(out=top8v, in_=masked)
                    idx8 = spool.tile([128, 8], U32, tag="idx8")
                    nc.vector.max_index(idx8, top8v, masked)
                    idx8f = spool.tile([128, TOPK], F32, tag="idx8f")
                    nc.vector.tensor_copy(idx8f, idx8)
                    w8 = spool.tile([128, TOPK], F32, tag="w8")
                    eqs = spool.tile([128, E], F32, tag="eqs")
                    for k in range(TOPK):
                        nc.vector.scalar_tensor_tensor(
                            out=eqs,
                            in0=iota_sb,
                            scalar=idx8f[:, k : k + 1],
                            in1=scores,
                            op0=mybir.AluOpType.is_equal,
                            op1=mybir.AluOpType.mult,
                            accum_out=w8[:, k : k + 1],
                        )
                    denom = spool.tile([128, 1], F32, tag="denom")
                    nc.vector.reduce_sum(denom, w8, axis=mybir.AxisListType.X)
                    rden = spool.tile([128, 1], F32, tag="rden")
                    nc.vector.reciprocal(rden, denom)
                    wout = spool.tile([128, TOPK], F32, tag="wout")
                    nc.vector.tensor_scalar(
                        wout, w8, rden, SCALE,
                        op0=mybir.AluOpType.mult, op1=mybir.AluOpType.mult,
                    )
                    nc.scalar.dma_start(out=out_idx[ds(t0, 128), :], in_=idx8)
                    nc.scalar.dma_start(out=out_w[ds(t0, 128), :], in_=wout)
    nc.finalize()
    return nc


def make_in_maps_f16(hidden_states, weight, e_score_correction_bias):
    x = np.ascontiguousarray(np.asarray(hidden_states, dtype=np.float32)).reshape(T, H)
    w = np.asarray(weight, dtype=np.float32)
    b = np.asarray(e_score_correction_bias, dtype=np.float32)
    ws = np.ascontiguousarray(w.T) * np.float32(WSCALE)
    whi = ws.astype(np.float16)
    wlo = (ws - whi.astype(np.float32)).astype(np.float16)
    biasb = np.ascontiguousarray(np.broadcast_to(b[None, :], (128, E)))
    iotab = np.ascontiguousarray(
        np.broadcast_to(np.arange(E, dtype=np.float32)[None, :], (128, E)))
    xt_full = x.T  # view
    in_maps = []
    for c in range(N_CORES):
        xs = np.ascontiguousarray(xt_full[:, c * TC : (c + 1) * TC]) * np.float32(XSCALE)
        xhi = xs.astype(np.float16)
        xlo = (xs - xhi.astype(np.float32)).astype(np.float16)
        in_maps.append({"xhi": xhi, "xlo": xlo, "whi": whi, "wlo": wlo,
                        "biasb": biasb, "iotab": iotab})
    return in_maps


_NC = None


def _get_nc():
    global _NC
    if _NC is None:
        _NC = build_nc_f16()
    return _NC


def make_in_maps(hidden_states, weight, e_score_correction_bias):
    x = np.ascontiguousarray(np.asarray(hidden_states, dtype=np.float32)).reshape(T, H)
    w = np.asarray(weight, dtype=np.float32)
    b = np.asarray(e_score_correction_bias, dtype=np.float32)
    wt = np.ascontiguousarray(w.T)
    biasb = np.ascontiguousarray(np.broadcast_to(b[None, :], (128, E)))
    iotab = np.ascontiguousarray(np.broadcast_to(np.arange(E, dtype=np.float32)[None, :], (128, E)))
    xt_full = x.T  # view
    in_maps = []
    for c in range(N_CORES):
        xt_c = np.ascontiguousarray(xt_full[:, c * TC : (c + 1) * TC])
        in_maps.append({"xt": xt_c, "wt": wt, "biasb": biasb, "iotab": iotab})
    return in_maps


def assemble(results):
    logits = np.concatenate([results[c]["out_logits"] for c in range(N_CORES)], axis=0)
    idx = np.concatenate([results[c]["out_idx"] for c in range(N_CORES)], axis=0).astype(np.int32)
    wts = np.concatenate([results[c]["out_w"] for c in range(N_CORES)], axis=0)
    return idx, wts, logits


make_in_maps_active = None  # set below


def kernel(hidden_states, weight, e_score_correction_bias):
    nc = _get_nc()
    in_maps = make_in_maps_active(hidden_states, weight, e_score_correction_bias)
    res = run_bass_kernel_spmd(nc, in_maps, list(range(N_CORES)))
    return assemble(res.results)


make_in_maps_active = make_in_maps_f16


# revision 12
# speedup vs baseline: 1.3050x; 1.0636x over previous
"""DeepSeek-V3 TopK router kernel for 8 Trainium2 NeuronCores.

Strategy (data/sequence parallel per sharding hint):
 - Shard the 16384 tokens across 8 cores (2048 tokens each); replicate the
   router weight + bias.
 - Host-side layout prep: x and w are pre-transposed so the contraction dim
   (hidden) lands on SBUF partitions; total DMA bytes are unchanged.
 - Device per core: 56 accumulating fp32r matmuls per 128-token tile
   (lhsT = x^T chunk [128h,128t] stationary, rhs = w^T chunk [128h,256e]
   moving, PSUM [128t,256e]); sigmoid on ScalarE; group top-2 / top-4 and
   masked top-8 via the DVE max8/max_index ops; weight gather via fused
   scalar_tensor_tensor (eq-mask * scores, reduced); normalize with DVE
   reciprocal.
"""

import sys

for _p in ("/opt/trn_rl_repo", "/root/.axon_site/_ro/trn_rl_repo"):
    if _p not in sys.path:
        sys.path.append(_p)

import numpy as np

import concourse.bass as bass
import concourse.bacc as bacc
import concourse.mybir as mybir
from concourse.bass import ds
from concourse.tile import TileContext
from concourse.bass_utils import run_bass_kernel_spmd

# Problem constants (hardcoded per contract)
T = 16384          # batch*seq = 4*4096
H = 7168           # hidden
E = 256            # experts
N_CORES = 8
TC = T // N_CORES  # tokens per core = 2048
KH = H // 128      # 56 contraction chunks
NG = 8             # expert groups
GS = E // NG       # group size = 32
TOPK = 8
SCALE = 2.5
TGROUP = 256       # tokens per DMA group (2 matmul subtiles)
CK = 8             # contraction chunks per DMA (k-chunking for overlap)
NCH = KH // CK     # 7 DMA chunks over the hidden dim

F32 = mybir.dt.float32
F32R = mybir.dt.float32r
U32 = mybir.dt.uint32


def build_nc(tc_tokens: int = TC, tgroup: int = TGROUP, debug: bool = False) -> bass.Bass:
    nc = bacc.Bacc(trn_type="TRN2")
    xt = nc.declare_dram_parameter("xt", [H, tc_tokens], F32, isOutput=False)
    wt = nc.declare_dram_parameter("wt", [H, E], F32, isOutput=False)
    biasb = nc.declare_dram_parameter("biasb", [128, E], F32, isOutput=False)
    iotab = nc.declare_dram_parameter("iotab", [128, E], F32, isOutput=False)
    out_logits = nc.declare_dram_parameter("out_logits", [tc_tokens, E], F32, isOutput=True)
    out_idx = nc.declare_dram_parameter("out_idx", [tc_tokens, TOPK], U32, isOutput=True)
    out_w = nc.declare_dram_parameter("out_w", [tc_tokens, TOPK], F32, isOutput=True)
    if debug:
        out_w8 = nc.declare_dram_parameter("out_w8", [tc_tokens, TOPK], F32, isOutput=True)
        out_den = nc.declare_dram_parameter("out_den", [tc_tokens, 2], F32, isOutput=True)
        out_eqs = nc.declare_dram_parameter("out_eqs", [tc_tokens, E], F32, isOutput=True)

    n_groups = tc_tokens // tgroup
    subs = tgroup // 128

    with TileContext(nc) as tc:
        with (
            tc.tile_pool(name="const", bufs=1) as cpool,
            tc.tile_pool(name="xs", bufs=2) as xpool,
            tc.tile_pool(name="ps", bufs=4, space="PSUM") as ppool,
            tc.tile_pool(name="work", bufs=2) as spool,
        ):
            wt_sb = cpool.tile([128, KH, E], F32)
            nc.sync.dma_start(out=wt_sb, in_=wt[:, :].rearrange("(k p) e -> p k e", p=128))
            bias_sb = cpool.tile([128, E], F32)
            nc.sync.dma_start(out=bias_sb, in_=biasb[:, :])
            iota_sb = cpool.tile([128, E], F32)
            nc.sync.dma_start(out=iota_sb, in_=iotab[:, :])
            # Dummy ldweights consuming wt_sb: folds the weight-DMA wait into
            # the PE's clock so the first real matmul carries only the x-tile
            # wait (the LDWEIGHTS ISA slot fits a single sync wait). The real
            # matmuls self-load their stationary operands, overwriting this.
            nc.tensor.ldweights(wt_sb[:, 0, 0:64].bitcast(mybir.dt.bfloat16))

            for g in range(n_groups):
                xg = xpool.tile([128, KH, tgroup], F32, tag="xg")
                nc.sync.dma_start(
                    out=xg,
                    in_=xt[:, ds(g * tgroup, tgroup)].rearrange("(k p) t -> p k t", p=128),
                )
                for sub in range(subs):
                    t0 = g * tgroup + sub * 128
                    lg = ppool.tile([128, E], F32, tag="lg")
                    for k in range(KH):
                        nc.tensor.matmul(
                            lg,
                            xg[:, k, ds(sub * 128, 128)],
                            wt_sb[:, k, :],
                            start=(k == 0),
                            stop=(k == KH - 1),
                        )
                    # router logits: PSUM -> SBUF -> DRAM
                    lsb = spool.tile([128, E], F32, tag="lsb")
                    nc.scalar.copy(lsb, lg)
                    nc.scalar.dma_start(out=out_logits[ds(t0, 128), :], in_=lsb)
                    scores = spool.tile([128, E], F32, tag="scores")
                    nc.scalar.activation(scores, lg, mybir.ActivationFunctionType.Sigmoid)
                    s4c = spool.tile([128, E], F32, tag="s4c")
                    nc.vector.tensor_add(s4c, scores, bias_sb)

                    # per-group top-2 -> group score
                    gtop = spool.tile([128, NG, 8], F32, tag="gtop")
                    for j in range(NG):
                        nc.vector.max(out=gtop[:, j, :], in_=s4c[:, ds(j * GS, GS)])
                    gsum = spool.tile([128, NG], F32, tag="gsum")
                    nc.vector.tensor_add(gsum, gtop[:, :, 0], gtop[:, :, 1])
                    # top-4 groups -> mask
                    gs8 = spool.tile([128, 8], F32, tag="gs8")
                    nc.vector.max(out=gs8, in_=gsum)
                    gmask = spool.tile([128, NG], F32, tag="gmask")
                    nc.vector.tensor_scalar(
                        gmask, gsum, gs8[:, 3:4], None, op0=mybir.AluOpType.is_ge
                    )
                    masked = spool.tile([128, E], F32, tag="masked")
                    for j in range(NG):
                        nc.vector.tensor_scalar_mul(
                            masked[:, ds(j * GS, GS)], s4c[:, ds(j * GS, GS)], gmask[:, j : j + 1]
                        )
                    # masked top-8 with indices
                    top8v = spool.tile([128, 8], F32, tag="top8v")
                    nc.vector.max(out=top8v, in_=masked)
                    idx8 = spool.tile([128, 8], U32, tag="idx8")
                    nc.vector.max_index(idx8, top8v, masked)
                    # gather scores at the top-8 positions by POSITION
                    # (iota == idx_k is a guaranteed one-hot; value-matching
                    # breaks when two experts tie bit-exactly)
                    idx8f = spool.tile([128, TOPK], F32, tag="idx8f")
                    nc.vector.tensor_copy(idx8f, idx8)
                    w8 = spool.tile([128, TOPK], F32, tag="w8")
                    eqs = spool.tile([128, E], F32, tag="eqs")
                    for k in range(TOPK):
                        nc.vector.scalar_tensor_tensor(
                            out=eqs,
                            in0=iota_sb,
                            scalar=idx8f[:, k : k + 1],
                            in1=scores,
                            op0=mybir.AluOpType.is_equal,
                            op1=mybir.AluOpType.mult,
                            accum_out=w8[:, k : k + 1],
                        )
                    denom = spool.tile([128, 1], F32, tag="denom")
                    nc.vector.reduce_sum(denom, w8, axis=mybir.AxisListType.X)
                    rden = spool.tile([128, 1], F32, tag="rden")
                    nc.vector.reciprocal(rden, denom)
                    wout = spool.tile([128, TOPK], F32, tag="wout")
                    nc.vector.tensor_scalar(
                        wout, w8, rden, SCALE,
                        op0=mybir.AluOpType.mult, op1=mybir.AluOpType.mult,
                    )
                    nc.scalar.dma_start(out=out_idx[ds(t0, 128), :], in_=idx8)
                    nc.scalar.dma_start(out=out_w[ds(t0, 128), :], in_=wout)
                    if debug:
                        nc.scalar.dma_start(out=out_w8[ds(t0, 128), :], in_=w8)
                        nc.scalar.dma_start(out=out_den[ds(t0, 128), 0:1], in_=denom)
                        nc.scalar.dma_start(out=out_den[ds(t0, 128), 1:2], in_=rden)
                        nc.scalar.dma_start(out=out_eqs[ds(t0, 128), :], in_=eqs)
    nc.finalize()
    return nc


F16 = mybir.dt.float16
XSCALE = 1024.0       # 2**10: keeps x_lo out of fp16 denormals
WSCALE = 8192.0       # 2**13: keeps w_lo out of fp16 denormals
DESCALE = 1.0 / (XSCALE * WSCALE)  # 2**-23, exact power of two


def build_nc_f16(tc_tokens: int = TC, tgroup: int = TGROUP, debug: bool = False) -> bass.Bass:
    """fp16 hi/lo 3-pass matmul variant: x' = x*2^10 = xhi+xlo (fp16),
    w' = w*2^13 = whi+wlo (fp16); logits' = xhi*whi + xlo*whi + xhi*wlo
    accumulated in fp32 PSUM; descale by 2^-23 (exact) on the way out.
    Error ~2^-21 per product: below fp32 summation noise, at 3x bf16-rate
    PE cost instead of 4x for native fp32."""
    nc = bacc.Bacc(trn_type="TRN2")
    xhi = nc.declare_dram_parameter("xhi", [H, tc_tokens], F16, isOutput=False)
    xlo = nc.declare_dram_parameter("xlo", [H, tc_tokens], F16, isOutput=False)
    whi = nc.declare_dram_parameter("whi", [H, E], F16, isOutput=False)
    wlo = nc.declare_dram_parameter("wlo", [H, E], F16, isOutput=False)
    biasb = nc.declare_dram_parameter("biasb", [128, E], F32, isOutput=False)
    iotab = nc.declare_dram_parameter("iotab", [128, E], F32, isOutput=False)
    out_logits = nc.declare_dram_parameter("out_logits", [tc_tokens, E], F32, isOutput=True)
    out_idx = nc.declare_dram_parameter("out_idx", [tc_tokens, TOPK], U32, isOutput=True)
    out_w = nc.declare_dram_parameter("out_w", [tc_tokens, TOPK], F32, isOutput=True)

    n_groups = tc_tokens // tgroup
    subs = tgroup // 128

    with TileContext(nc) as tc:
        with (
            tc.tile_pool(name="const", bufs=1) as cpool,
            tc.tile_pool(name="xs", bufs=2) as xpool,
            tc.tile_pool(name="ps", bufs=4, space="PSUM") as ppool,
            tc.tile_pool(name="work", bufs=2) as spool,
        ):
            # K is processed in chunks so matmuls can start as soon as the
            # first slices of the weights and x land, instead of waiting for
            # whole-tensor DMAs (~15 MB) to finish.
            whi_ch, wlo_ch = [], []
            for c in range(NCH):
                wh = cpool.tile([128, CK, E], F16, tag=f"whi{c}")
                nc.sync.dma_start(
                    out=wh,
                    in_=whi[ds(c * CK * 128, CK * 128), :].rearrange("(k p) e -> p k e", p=128))
                wl = cpool.tile([128, CK, E], F16, tag=f"wlo{c}")
                nc.sync.dma_start(
                    out=wl,
                    in_=wlo[ds(c * CK * 128, CK * 128), :].rearrange("(k p) e -> p k e", p=128))
                whi_ch.append(wh)
                wlo_ch.append(wl)
            bias_sb = cpool.tile([128, E], F32)
            nc.sync.dma_start(out=bias_sb, in_=biasb[:, :])
            iota_sb = cpool.tile([128, E], F32)
            nc.sync.dma_start(out=iota_sb, in_=iotab[:, :])

            for g in range(n_groups):
                xghi_ch, xglo_ch = [], []
                for c in range(NCH):
                    xh = xpool.tile([128, CK, tgroup], F16, tag=f"xghi{c}")
                    nc.sync.dma_start(
                        out=xh,
                        in_=xhi[ds(c * CK * 128, CK * 128), ds(g * tgroup, tgroup)]
                        .rearrange("(k p) t -> p k t", p=128))
                    xl = xpool.tile([128, CK, tgroup], F16, tag=f"xglo{c}")
                    nc.sync.dma_start(
                        out=xl,
                        in_=xlo[ds(c * CK * 128, CK * 128), ds(g * tgroup, tgroup)]
                        .rearrange("(k p) t -> p k t", p=128))
                    xghi_ch.append(xh)
                    xglo_ch.append(xl)
                for sub in range(subs):
                    t0 = g * tgroup + sub * 128
                    ts_ = ds(sub * 128, 128)
                    lg = ppool.tile([128, E], F32, tag="lg")
                    for k in range(KH):
                        c, kc = divmod(k, CK)
                        nc.tensor.matmul(lg, xghi_ch[c][:, kc, ts_], whi_ch[c][:, kc, :],
                                         start=(k == 0), stop=False)
                        nc.tensor.matmul(lg, xghi_ch[c][:, kc, ts_], wlo_ch[c][:, kc, :],
                                         start=False, stop=False)
                        nc.tensor.matmul(lg, xglo_ch[c][:, kc, ts_], whi_ch[c][:, kc, :],
                                         start=False, stop=(k == KH - 1))
                    # descaled router logits: PSUM -> SBUF -> DRAM
                    lsb = spool.tile([128, E], F32, tag="lsb")
                    nc.scalar.mul(lsb, lg, DESCALE)
                    nc.scalar.dma_start(out=out_logits[ds(t0, 128), :], in_=lsb)
                    scores = spool.tile([128, E], F32, tag="scores")
                    nc.scalar.activation(scores, lg, mybir.ActivationFunctionType.Sigmoid,
                                         scale=DESCALE)
                    s4c = spool.tile([128, E], F32, tag="s4c")
                    nc.vector.tensor_add(s4c, scores, bias_sb)

                    gtop = spool.tile([128, NG, 8], F32, tag="gtop")
                    for j in range(NG):
                        nc.vector.max(out=gtop[:, j, :], in_=s4c[:, ds(j * GS, GS)])
                    gsum = spool.tile([128, NG], F32, tag="gsum")
                    nc.vector.tensor_add(gsum, gtop[:, :, 0], gtop[:, :, 1])
                    gs8 = spool.tile([128, 8], F32, tag="gs8")
                    nc.vector.max(out=gs8, in_=gsum)
                    gmask = spool.tile([128, NG], F32, tag="gmask")
                    nc.vector.tensor_scalar(
                        gmask, gsum, gs8[:, 3:4], None, op0=mybir.AluOpType.is_ge
                    )
                    masked = spool.tile([128, E], F32, tag="masked")
                    for j in range(NG):
                        nc.vector.tensor_scalar_mul(
                            masked[:, ds(j * GS, GS)], s4c[:, ds(j * GS, GS)], gmask[:, j : j + 1]
                        )
                    top8v = spool.tile([128, 8], F32, tag="top8v")
                    nc.vector.max(out=top8v, in_=masked)
                    idx8 = spool.tile([128, 8], U32, tag="idx8")
                    nc.vector.max_index(idx8, top8v, masked)
                    idx8f = spool.tile([128, TOPK], F32, tag="idx8f")
                    nc.vector.tensor_copy(idx8f, idx8)
                    w8 = spool.tile([128, TOPK], F32, tag="w8")
                    eqs = spool.tile([128, E], F32, tag="eqs")
                    for k in range(TOPK):
                        nc.vector.scalar_tensor_tensor(
                            out=eqs,
                            in0=iota_sb,
                            scalar=idx8f[:, k : k + 1],
                            in1=scores,
                            op0=mybir.AluOpType.is_equal,
                            op1=mybir.AluOpType.mult,
                            accum_out=w8[:, k : k + 1],
                        )
                    denom = spool.tile([128, 1], F32, tag="denom")
                    nc.vector.reduce_sum(denom, w8, axis=mybir.AxisListType.X)
                    rden = spool.tile([128, 1], F32, tag="rden")
                    nc.vector.reciprocal(rden, denom)
                    wout = spool.tile([128, TOPK], F32, tag="wout")
                    nc.vector.tensor_scalar(
                        wout, w8, rden, SCALE,
                        op0=mybir.AluOpType.mult, op1=mybir.AluOpType.mult,
                    )
                    nc.scalar.dma_start(out=out_idx[ds(t0, 128), :], in_=idx8)
                    nc.scalar.dma_start(out=out_w[ds(t0, 128), :], in_=wout)
    nc.finalize()
    return nc


def make_in_maps_f16(hidden_states, weight, e_score_correction_bias):
    x = np.ascontiguousarray(np.asarray(hidden_states, dtype=np.float32)).reshape(T, H)
    w = np.asarray(weight, dtype=np.float32)
    b = np.asarray(e_score_correction_bias, dtype=np.float32)
    ws = np.ascontiguousarray(w.T) * np.float32(WSCALE)
    whi = ws.astype(np.float16)
    wlo = (ws - whi.astype(np.float32)).astype(np.float16)
    biasb = np.ascontiguousarray(np.broadcast_to(b[None, :], (128, E)))
    iotab = np.ascontiguousarray(
        np.broadcast_to(np.arange(E, dtype=np.float32)[None, :], (128, E)))
    xt_full = x.T  # view
    in_maps = []
    for c in range(N_CORES):
        xs = np.ascontiguousarray(xt_full[:, c * TC : (c + 1) * TC]) * np.float32(XSCALE)
        xhi = xs.astype(np.float16)
        xlo = (xs - xhi.astype(np.float32)).astype(np.float16)
        in_maps.append({"xhi": xhi, "xlo": xlo, "whi": whi, "wlo": wlo,
                        "biasb": biasb, "iotab": iotab})
    return in_maps


_NC = None


def _get_nc():
    global _NC
    if _NC is None:
        _NC = build_nc_f16()
    return _NC


def make_in_maps(hidden_states, weight, e_score_correction_bias):
    x = np.ascontiguousarray(np.asarray(hidden_states, dtype=np.float32)).reshape(T, H)
    w = np.asarray(weight, dtype=np.float32)
    b = np.asarray(e_score_correction_bias, dtype=np.float32)
    wt = np.ascontiguousarray(w.T)
    biasb = np.ascontiguousarray(np.broadcast_to(b[None, :], (128, E)))
    iotab = np.ascontiguousarray(np.broadcast_to(np.arange(E, dtype=np.float32)[None, :], (128, E)))
    xt_full = x.T  # view
    in_maps = []
    for c in range(N_CORES):
        xt_c = np.ascontiguousarray(xt_full[:, c * TC : (c + 1) * TC])
        in_maps.append({"xt": xt_c, "wt": wt, "biasb": biasb, "iotab": iotab})
    return in_maps


def assemble(results):
    logits = np.concatenate([results[c]["out_logits"] for c in range(N_CORES)], axis=0)
    idx = np.concatenate([results[c]["out_idx"] for c in range(N_CORES)], axis=0).astype(np.int32)
    wts = np.concatenate([results[c]["out_w"] for c in range(N_CORES)], axis=0)
    return idx, wts, logits


make_in_maps_active = None  # set below


def kernel(hidden_states, weight, e_score_correction_bias):
    nc = _get_nc()
    in_maps = make_in_maps_active(hidden_states, weight, e_score_correction_bias)
    res = run_bass_kernel_spmd(nc, in_maps, list(range(N_CORES)))
    return assemble(res.results)


make_in_maps_active = make_in_maps_f16
